# revision 45
# baseline (speedup 1.0000x reference)
"""Trainium2 Bass kernel for nn_DecoderCrossMSA (Swin-style shifted-window
cross-attention).

Strategy: data-parallel over batch (8 batches -> 8 cores). Host prepares, per
core, feature-major window-ordered activations (token axis permuted so every
8x8 shifted window is a contiguous 64-token run; roll folded into the
permutation). Device per 512-token block:
  - Q/K projections (bf16, Q pre-scaled by 1/sqrt(32), biases in evac stage),
  - V projections emitted token-major into per-pair [128, sc|sh] tiles,
  - windowed attention, software-pipelined across window pairs:
      S^T per (head, window) into a 4-bank PSUM tile (bank == h%4 == PE
      row-group, so concurrent array-tiled matmuls never collide on a
      (bank, partition) pair), exp on ACT, multiplicative exp-bias/mask
      table on DVE, replicated row-sums via ones-matmuls into two 1-bank
      proj-pool tiles, and softmax normalization DEFERRED: AV runs on
      unnormalized weights into a single packed 2-bank tile (bank ==
      window half) and the PSUM->SBUF evacuation is a tensor_tensor
      multiply by the reciprocal row-sums,
  - output projections with V-biases folded in host-side, emitted as
    deferred per-(mo,out) groups interleaved between the next block's pair
    stages so the in-order PE stream never starves near block boundaries.
The pair loop is emitted with a 1-deep skew (S of pair p before row-sum/AV
of pair p-1) so PE always has independent work queued while the ACT/DVE
softmax stages of the newest pair are in flight. PSUM pools sum to exactly
8 banks (proj 2 + S^T 4 + AV 2). All DMA rides the SP HWDGE queue, ordered
so block 0's weights/inputs land first; outputs stream out per 128-channel
chunk as soon as each evacuation finishes.
"""

import numpy as np
import ml_dtypes

EMB = 512
HEADS = 16
WS = 8
B = 8
HW = 64
N = HW * HW
EH = EMB // HEADS          # 32
WN = HW // WS              # 8
SHIFT = WS // 2            # 4
WT = WS * WS               # 64 tokens per window
NCORES = 8
NBLK = 8                   # token blocks per core (512 tokens each)
BLKT = N // NBLK           # 512
MASK_NEG = -30000.0

_bf16 = ml_dtypes.bfloat16


def _build_perm(shift):
    """perm[t] = token index n for window-ordered position t.

    t = ((i*WN + j) * WT) + (w1*WS + w2); grid row = (WS*i + w1 + shift) mod
    HW, col = (WS*j + w2 + shift) mod HW. Inputs are read through the rolled
    grid (shift=SHIFT); outputs are written back WITHOUT inverting the roll
    (shift=0) — the reference's _unwindow does not undo the roll.
    """
    i, j, w1, w2 = np.meshgrid(
        np.arange(WN), np.arange(WN), np.arange(WS), np.arange(WS), indexing="ij"
    )
    r = (WS * i + w1 + shift) % HW
    c = (WS * j + w2 + shift) % HW
    return (r * HW + c).reshape(-1)


_PERM = _build_perm(SHIFT)
_OPERM = _build_perm(0)

# Reference splits EMB as (e H): head h lives on strided channels e*HEADS+h.
# Permute projection out-channels so head h is the contiguous block h*EH..:
# new channel h*EH+e = old channel e*HEADS+h.
_RHO = np.array([e * HEADS + h for h in range(HEADS) for e in range(EH)])


def _pair_tables(pos_emb):
    """4 pair-type tables [128, 16*64] bf16 of exp(T)^T, head-replicated.

    T[q, k] = pos_bias[q, k] (+ row mask if window-row i == WN-1)
                         (+ col mask if window-col j == WN-1).
    Table rows = k (2 windows stacked: first window of pair rows 0:64, second
    rows 64:128), free = (16 head slots, 64 q) — all head slots identical.
    pair p = windows (2p, 2p+1): second window is col-masked iff p % 4 == 3;
    both windows row-masked iff p // 4 == WN - 1.
    """
    idx = np.array([[x, y] for x in range(WS) for y in range(WS)])
    rel = idx[None, :, :] - idx[:, None, :] + WS - 1
    bias = pos_emb[rel[:, :, 0], rel[:, :, 1]].astype(np.float64)

    m = np.zeros((WT, WT), dtype=np.float64)
    s = WS * (WS // 2)
    m[-s:, :-s] = MASK_NEG
    m[:-s, -s:] = MASK_NEG
    r = WT // WS
    col = m.reshape(r, WS, r, WS).transpose(1, 0, 3, 2).reshape(WT, WT)

    t0 = bias
    t1 = bias + m          # row-masked  (i == 7)
    t2 = bias + col        # col-masked  (j == 7)
    t3 = bias + m + col    # corner

    def pair_tab(ta, tb):
        ea = np.exp(ta).T    # [k, q]
        eb = np.exp(tb).T
        stk = np.concatenate([ea, eb], axis=0)           # [128, 64]
        rep = np.tile(stk, (1, HEADS))                   # [128, 16*64]
        return rep.astype(_bf16)

    # pair types: (normal,normal), (normal,colmask), (rowmask,rowmask),
    # (rowmask,corner)
    return np.stack([
        pair_tab(t0, t0),
        pair_tab(t0, t2),
        pair_tab(t1, t1),
        pair_tab(t1, t3),
    ])


def _pair_type(p):
    row = (p // 4) == WN - 1      # window-row i == 7
    colm = (p % 4) == 3           # second window j == 7
    return (2 if row else 0) + (1 if colm else 0)


def _slot(h):
    """pa/st free-dim slot base for head h: (h%4)*256 + (h//4)*64."""
    return (h % 4) * 256 + (h // 4) * 64


def _build_bass():
    import concourse.mybir as mybir
    from concourse import bacc
    from concourse.tile import TileContext

    fp32 = mybir.dt.float32
    bf16 = mybir.dt.bfloat16
    AF = mybir.ActivationFunctionType
    ALU = mybir.AluOpType

    nc = bacc.Bacc()

    # ---- DRAM parameters (per-core) ----
    d_in = {}
    for name in ("cw", "sw", "scw", "shw"):
        d_in[name] = nc.declare_dram_parameter(name, [EMB, N], bf16, isOutput=False)
    for name in ("w1t", "w2t", "wsct", "wsht", "wsot", "wshot"):
        d_in[name] = nc.declare_dram_parameter(name, [EMB, EMB], bf16, isOutput=False)
    for name in ("b1r", "b2r", "bsor", "bshor"):
        d_in[name] = nc.declare_dram_parameter(name, [128, 4], fp32, isOutput=False)
    d_in["ptab"] = nc.declare_dram_parameter(
        "ptab", [4, 128, HEADS * WT], bf16, isOutput=False
    )
    d_in["onesc"] = nc.declare_dram_parameter("onesc", [128, 32], bf16, isOutput=False)
    # outputs in bf16: halves the output DMA volume on the serial DMA
    # engines (host upcasts to fp32 after the gather)
    yso = nc.declare_dram_parameter("yso", [EMB, N], bf16, isOutput=True)
    ysho = nc.declare_dram_parameter("ysho", [EMB, N], bf16, isOutput=True)
    y_out = {"sc": yso, "sh": ysho}

    QTOK = 2 * BLKT            # tokens per input quarter-tile

    with TileContext(nc) as tc:
        with (
            tc.tile_pool(name="const", bufs=1) as cpool,
            tc.tile_pool(name="xq", bufs=2) as xqpool,
            tc.tile_pool(name="cs", bufs=2) as cspool,
            tc.tile_pool(name="v", bufs=8) as vpool,
            tc.tile_pool(name="pe", bufs=2) as pepool,
            tc.tile_pool(name="pa", bufs=2) as papool,
            tc.tile_pool(name="rd", bufs=2) as rdpool,
            tc.tile_pool(name="o", bufs=2) as opool,
            tc.tile_pool(name="y", bufs=2) as ypool,
            tc.tile_pool(name="projps", bufs=2, space="PSUM") as projps,
            tc.tile_pool(name="stps", bufs=1, space="PSUM") as stps,
            tc.tile_pool(name="avps", bufs=1, space="PSUM") as avps,
        ):
            # ---- PE p-state warmup: the cost model runs PE at half rate
            # for the first ~3us after it leaves idle; burn that ramp on
            # dummy matmuls during the otherwise-dead initial DMA wait so
            # the real projections start at full rate ----
            scratch = cpool.tile([128, BLKT], bf16, tag="scratch")
            nc.gpsimd.memset(scratch[:], 0)
            wps = projps.tile([128, BLKT], fp32, tag="proj", name="wps")
            for _ in range(14):
                nc.tensor.matmul(wps[:], lhsT=scratch[:, 0:128],
                                 rhs=scratch[:], start=True, stop=True)

            # ---- constants + inputs, ordered so block 0 can start ASAP ----
            wts = {}

            def load_weight(name, eng=None):
                # one [128, 4*512] tile per weight, k-chunks along free
                t = cpool.tile([128, 4 * EMB], bf16, tag=name)
                (eng or nc.sync).dma_start(
                    t[:].rearrange("p (k e) -> p k e", k=4),
                    d_in[name][:].rearrange("(k p) e -> p k e", k=4),
                )
                wts[name] = t

            def wslice(name, k, col):
                return wts[name][:, k * EMB + col.start:k * EMB + col.stop]

            bias_t = {}

            def load_bias(name, eng=None):
                t = cpool.tile([128, 4], fp32, tag=name)
                (eng or nc.sync).dma_start(t[:], d_in[name][:])
                bias_t[name] = t

            # first-block tiles for cw/sw (one DMA each), loaded first
            x0 = {}
            load_weight("w1t")
            for tname in ("cw", "sw"):
                if tname == "sw":
                    load_weight("w2t")
                t = cpool.tile([128, 4 * BLKT], bf16, tag=f"x0_{tname}")
                nc.sync.dma_start(
                    t[:].rearrange("p (k c) -> p k c", k=4),
                    d_in[tname][:, 0:BLKT].rearrange("(k p) c -> p k c", k=4),
                )
                x0[tname] = t
            load_bias("b1r")
            load_bias("b2r")
            load_weight("wsct")
            load_weight("wsht")

            # input quarter-tiles: xq[t][k][q] covers blocks 2q, 2q+1
            xq = {t: [[None] * 4 for _ in range(4)]
                  for t in ("cw", "sw", "scw", "shw")}

            def load_quarter(tname, q):
                for k in range(4):
                    t = xqpool.tile([128, QTOK], bf16, tag=f"xq_{tname}_{k}")
                    nc.sync.dma_start(
                        t[:],
                        d_in[tname][k * 128:(k + 1) * 128,
                                    q * QTOK:(q + 1) * QTOK],
                    )
                    xq[tname][k][q] = t

            load_quarter("scw", 0)
            load_quarter("shw", 0)
            ptab_t = []
            for i in range(4):
                t = cpool.tile([128, HEADS * WT], bf16, tag=f"ptab{i}")
                nc.sync.dma_start(t[:], d_in["ptab"][i])
                ptab_t.append(t)
            ones_t = cpool.tile([128, 32], bf16, tag="onesc")
            nc.sync.dma_start(ones_t[:], d_in["onesc"][:])
            # prime the ACT exp table set so the one-time ~2.7us table load
            # doesn't serialize the first pair's softmax
            warm = cpool.tile([128, 32], bf16, tag="warm")
            nc.scalar.activation(warm[:], ones_t[:], AF.Exp)
            load_weight("wsot")
            load_weight("wshot")
            load_bias("bsor")
            load_bias("bshor")
            load_quarter("cw", 0)
            load_quarter("sw", 0)
            for q in range(1, 4):
                for tname in ("cw", "sw", "scw", "shw"):
                    load_quarter(tname, q)

            prev = None      # pending pair ctx for the skewed tail stages
            prev_blk = None  # pending block ctx for the output projection
            oq = []          # deferred O-projection group emitters

            def emit_pair_tail(ctx):
                """Row-sum matmuls, reciprocal, AV matmuls, and the fused
                normalize+scatter evacuation for a finished pair.

                PSUM collision discipline (concurrent array-tiled matmuls
                must differ in bank or partition range): the two row-sum
                tiles are separate banks (one per window half) with four
                32-partition-disjoint matmuls each; the AV tile has
                bank == window half, partitions (h%4)*32 within a bank.
                """
                pa, v_t, osc, osh, t0 = ctx
                # ---- replicated row-sums, one 1-bank tile per window ----
                rd = rdpool.tile([128, 512], fp32, tag="rd")
                for wi in range(2):
                    sl = slice(wi * 64, wi * 64 + 64)
                    dd = projps.tile([128, BLKT], fp32, tag="proj")
                    for r in range(4):
                        nc.tensor.matmul(
                            dd[r * 32:(r + 1) * 32, 0:256],
                            lhsT=ones_t[sl, :],
                            rhs=pa[sl, r * 256:(r + 1) * 256],
                            start=True, stop=True,
                            tile_position=(wi * 64, r * 32),
                        )
                    nc.vector.reciprocal(
                        rd[:, wi * 256:(wi + 1) * 256], dd[:, 0:256]
                    )

                # ---- AV matmuls (unnormalized weights) ----
                # av free = wi*512 + tensor*256 + m*64 + q  (bank = wi)
                av = avps.tile([128, 1024], fp32, tag="av")
                for half in range(2):
                    for h in range(HEADS):
                        m, r = h // 4, (h % 4) * 32
                        s0 = _slot(h)
                        for wi in range(2):
                            sl = slice(wi * 64, wi * 64 + 64)
                            f0 = wi * 512 + half * 256 + m * 64
                            nc.tensor.matmul(
                                av[r:r + 32, f0:f0 + 64],
                                lhsT=v_t[sl, half * 512 + h * 32:
                                         half * 512 + (h + 1) * 32],
                                rhs=pa[sl, s0:s0 + 64],
                                start=True, stop=True,
                                tile_position=(wi * 64, r),
                            )

                # ---- fused normalize + scatter: O = av * (1/rowsum) ----
                av_v = av[:].rearrange("p (w h2 m q) -> p h2 w m q",
                                       w=2, h2=2, m=4)
                rdv = rd[:].rearrange("p (w m q) -> p w m q", w=2, m=4)
                for half, o_t in ((0, osc), (1, osh)):
                    dst = o_t[:].rearrange("p (m t) -> p m t", m=4)
                    dst = dst[:, :, t0:t0 + 128].rearrange(
                        "p m (w q) -> p w m q", w=2
                    )
                    nc.vector.tensor_tensor(
                        dst, av_v[:, half], rdv, ALU.mult
                    )

            def oproj_groups(ctx, alt_evac=False):
                """Output-projection (mo, out) group emitters, one per call,
                so the pair loop can interleave them between pair tails."""
                osc, osh, c0 = ctx
                ys = {}

                def emit_group(key, o_t, wname, bname, mo,
                               alt_evac=False):
                    if key not in ys:
                        ys[key] = ypool.tile([128, 4 * BLKT], bf16,
                                             tag=f"y_{key}", name=f"y_{key}")
                    y_sb = ys[key]
                    ps = projps.tile([128, BLKT], fp32, tag="proj")
                    for k in range(4):
                        nc.tensor.matmul(
                            ps[:],
                            lhsT=wslice(wname, k,
                                        slice(mo * 128, (mo + 1) * 128)),
                            rhs=o_t[:, k * BLKT:(k + 1) * BLKT],
                            start=(k == 0),
                            stop=(k == 3),
                        )
                    if alt_evac and (mo + (key == "sh")) % 2 == 1:
                        nc.vector.tensor_scalar_add(
                            y_sb[:, mo * BLKT:(mo + 1) * BLKT], ps[:],
                            bias_t[bname][:, mo:mo + 1],
                        )
                    else:
                        nc.scalar.activation(
                            y_sb[:, mo * BLKT:(mo + 1) * BLKT], ps[:],
                            AF.Identity, bias=bias_t[bname][:, mo:mo + 1],
                        )
                    nc.sync.dma_start(
                        y_out[key][mo * 128:(mo + 1) * 128, c0:c0 + BLKT],
                        y_sb[:, mo * BLKT:(mo + 1) * BLKT],
                    )

                for mo in range(4):
                    for key, o_t, wname, bname in (
                        ("sc", osc, "wsot", "bsor"),
                        ("sh", osh, "wshot", "bshor"),
                    ):
                        yield (lambda a=key, b=o_t, c=wname, d=bname,
                               e=mo: emit_group(a, b, c, d, e, alt_evac))

            def emit_oproj(ctx, alt_evac=False):
                for g in oproj_groups(ctx, alt_evac):
                    g()

            for blk in range(NBLK):
                c0 = blk * BLKT
                q, qo = blk // 2, (blk % 2) * BLKT

                # ---- Q/K projections (feature-major) ----
                cs = {}
                for tname, wname, bname in (
                    ("cw", "w1t", "b1r"), ("sw", "w2t", "b2r")
                ):
                    cs[tname] = []
                    for m in range(4):
                        ps = projps.tile([128, BLKT], fp32, tag="proj")
                        for k in range(4):
                            nc.tensor.matmul(
                                ps[:],
                                lhsT=wslice(wname, k,
                                            slice(m * 128, (m + 1) * 128)),
                                rhs=(x0[tname][:, k * BLKT:(k + 1) * BLKT]
                                     if blk == 0 else
                                     xq[tname][k][q][:, qo:qo + BLKT]),
                                start=(k == 0),
                                stop=(k == 3),
                            )
                        out = cspool.tile([128, BLKT], bf16, tag=f"cs_{tname}_{m}")
                        nc.scalar.activation(
                            out[:], ps[:], AF.Identity,
                            bias=bias_t[bname][:, m:m + 1],
                        )
                        cs[tname].append(out)
                cT, sT = cs["cw"], cs["sw"]

                # ---- S/exp/pa stage emitter (used by the pair loop; for
                # block 0, pair 0 is emitted early to span the window where
                # PE has finished Q/K but the V inputs are still in flight
                # on the serial DMA chain) ----
                def emit_s(p):
                    pg = blk * 4 + p
                    ptype = _pair_type(pg)
                    t0 = p * 128
                    # S^T psum: 4 banks; head h lands in bank h%4 == its PE
                    # row-group, so concurrent row-tiled matmuls never share
                    # a (bank, partition) pair.
                    st = stps.tile([128, 2048], fp32, tag="st", name="st")
                    for h in range(HEADS):
                        m, r = h // 4, (h % 4) * 32
                        s0 = (h % 4) * 512 + (h // 4) * 64
                        for wi in range(2):
                            o0 = t0 + wi * WT
                            nc.tensor.matmul(
                                st[wi * 64:wi * 64 + 64, s0:s0 + 64],
                                lhsT=sT[m][r:r + 32, o0:o0 + WT],
                                rhs=cT[m][r:r + 32, o0:o0 + WT],
                                start=True, stop=True,
                                tile_position=(r, wi * 64),
                            )
                    # compact [128, 4, 4, 64] view of the used st slots
                    st_v = st[:].rearrange(
                        "p (b s q) -> p b s q", b=4, s=8, q=WT
                    )[:, :, 0:4, :]
                    pe = pepool.tile([128, HEADS * WT], bf16, tag="pe",
                                     name="pe")
                    pe_v = pe[:].rearrange("p (b s q) -> p b s q", b=4, s=4,
                                           q=WT)
                    nc.scalar.activation(pe_v, st_v, AF.Exp)
                    pa = papool.tile([128, HEADS * WT], bf16, tag="pa",
                                     name="pa")
                    nc.vector.tensor_tensor(
                        pa[:], pe[:], ptab_t[ptype][:], ALU.mult
                    )
                    return pa, t0

                pre = emit_s(0) if blk == 0 else None

                # ---- V projections (token-major), [128, Vsc|Vsh] per pair ----
                v_ts = []
                for p in range(4):
                    t0 = p * 128
                    v_t = vpool.tile([128, 2 * EMB], bf16, tag="v")
                    for half, tname, wname in (
                        (0, "scw", "wsct"), (1, "shw", "wsht")
                    ):
                        ps = projps.tile([128, EMB], fp32, tag="proj")
                        for k in range(4):
                            nc.tensor.matmul(
                                ps[:],
                                lhsT=xq[tname][k][q][:, qo + t0:qo + t0 + 128],
                                rhs=wslice(wname, k, slice(0, EMB)),
                                start=(k == 0),
                                stop=(k == 3),
                            )
                        nc.scalar.activation(
                            v_t[:, half * EMB:(half + 1) * EMB], ps[:],
                            AF.Identity,
                        )
                    v_ts.append(v_t)

                osc = opool.tile([128, 4 * BLKT], bf16, tag="osc")
                osh = opool.tile([128, 4 * BLKT], bf16, tag="osh")

                # ---- attention pairs, skew-1 software pipeline ----
                for p in range(4):
                    if pre is not None and p == 0:
                        pa, t0 = pre
                        pre = None
                    else:
                        pa, t0 = emit_s(p)
                    if prev is not None:
                        emit_pair_tail(prev)
                    prev = (pa, v_ts[p], osc, osh, t0)
                    if p == 0 and prev_blk is not None:
                        oq.extend(oproj_groups(prev_blk))
                        prev_blk = None
                    # two deferred O-proj groups between pair stages keep the
                    # in-order PE stream fed while DVE drains the newest tail
                    # (at the last block's first stage, hold two back so the
                    # final pair tails still have PE filler)
                    if not (blk == NBLK - 1 and p == 0):
                        for _ in range(2):
                            if oq:
                                oq.pop(0)()
                prev_blk = (osc, osh, c0)

            emit_pair_tail(prev)
            while oq:
                oq.pop(0)()
            emit_oproj(prev_blk, alt_evac=True)
    nc.compile()
    return nc


_NC_CACHE = {}
LAST_RESULT = None


def make_in_maps(content, style, scale, shift, W1, b1, W2, b2, Wsc, bsc,
                 Wsh, bsh, Wso, bso, Wsho, bsho, pos_emb):
    inv = 1.0 / np.sqrt(EMB / HEADS)
    f32 = np.float32

    # head-contiguous channel permutation on projection out-channels (_RHO);
    # inverted on the output-projection in-channels.
    w1t = (np.asarray(W1, f32)[_RHO].T * inv).astype(_bf16)  # [e_in, e_out], scaled
    w2t = np.asarray(W2, f32)[_RHO].T.astype(_bf16)
    wsct = np.asarray(Wsc, f32)[_RHO].T.astype(_bf16)
    wsht = np.asarray(Wsh, f32)[_RHO].T.astype(_bf16)
    # device O-row order is H-major (h*EH+e), so the output projections are
    # NOT channel-permuted.
    wsot = np.asarray(Wso, f32).T.astype(_bf16)
    wshot = np.asarray(Wsho, f32).T.astype(_bf16)
    b1r = (np.asarray(b1, f32)[_RHO] * inv).reshape(4, 128).T.copy()
    b2r = np.asarray(b2, f32)[_RHO].reshape(4, 128).T.copy()
    # V biases folded into output-projection biases; V channels reach the
    # output projection in H-major order, hence bsc[_RHO].
    bso2 = np.asarray(Wso, f32) @ np.asarray(bsc, f32)[_RHO] + np.asarray(bso, f32)
    bsho2 = (np.asarray(Wsho, f32) @ np.asarray(bsh, f32)[_RHO]
             + np.asarray(bsho, f32))
    bsor = bso2.reshape(4, 128).T.copy()
    bshor = bsho2.reshape(4, 128).T.copy()
    ptab = _pair_tables(np.asarray(pos_emb, f32))
    onesc = np.ones((128, 32), dtype=_bf16)

    common = dict(
        w1t=w1t, w2t=w2t, wsct=wsct, wsht=wsht, wsot=wsot, wshot=wshot,
        b1r=b1r, b2r=b2r, bsor=bsor, bshor=bshor, ptab=ptab, onesc=onesc,
    )
    in_maps = []
    for b in range(NCORES):
        m = dict(common)
        for name, full in (("cw", content), ("sw", style),
                           ("scw", scale), ("shw", shift)):
            x = np.asarray(full[b], f32)[_PERM]           # [N, EMB] window order
            m[name] = np.ascontiguousarray(x.T).astype(_bf16)
        in_maps.append(m)
    return in_maps


def kernel(**inputs):
    global LAST_RESULT
    from concourse.bass_utils import run_bass_kernel_spmd

    in_maps = make_in_maps(**inputs)

    if "nc" not in _NC_CACHE:
        _NC_CACHE["nc"] = _build_bass()
    res = run_bass_kernel_spmd(_NC_CACHE["nc"], in_maps, list(range(NCORES)))
    LAST_RESULT = res

    out_sc = np.empty((B, N, EMB), np.float32)
    out_sh = np.empty((B, N, EMB), np.float32)
    for b in range(NCORES):
        out_sc[b][_OPERM] = res.results[b]["yso"].T
        out_sh[b][_OPERM] = res.results[b]["ysho"].T
    return out_sc, out_sh
